# revision 12
# baseline (speedup 1.0000x reference)
"""Trainium2 Bass kernel for the BezierSurv censor-margin loss.

Math: for each row b of sim [B, C*S] (C=16 classes, S=256 samples):
  pos/neg masks over the C class segments are fully determined by
  (label[b], censor[b]); both masked means are linear in the per-class
  segment sums.  So
     loss_term[b] = relu(MARGIN - pos_mean + neg_mean)
                  = relu(MARGIN - sum_c W[b,c] * class_sum[b,c])
  with W[b,c] = pos_mask/pos_cnt - neg_mask/neg_cnt (host-precomputed
  [B,16] f32 — tiny), and class_sum the [B,16] segment-reduce of sim —
  the only memory-bound work (256 MiB of HBM reads).

Distribution: pure data parallel over 8 NeuronCores, 2048 rows each.
Per core: 16 row-tiles of [128, 4096]; per tile one DVE 3D-AP segment
reduce into a persistent [128, T*C] cs_all.  The margin dot product
(cs_all * W, 3D reduce) and the ScalarE relu run once at the end, so the
steady-state DVE work per tile (wait + one reduce, ~4.6us) stays under
the 5.8us DMA time per 2 MiB tile.  The last two tiles stream in four
1024-column chunks so the tail reduce overlaps the final DMAs.
Output: per-row relu terms [128,16]; final mean on host.

Raw Bass (no TileContext): explicit 4-buffer DMA pipeline with one
semaphore per buffer slot (unambiguous completion ordering).  SP issues
HWDGE DMAs; DVE reduces; ACT does the relu.
"""

import sys

import numpy as np

for _p in ("/opt/trn_rl_repo",):
    if _p not in sys.path:
        sys.path.insert(0, _p)

from contextlib import ExitStack

import concourse.bass as bass
import concourse.mybir as mybir
from concourse.bass_utils import run_bass_kernel_spmd

MARGIN = 0.1
B = 16384
C = 16
S = 256
CS = C * S
N_CORES = 8
RPC = B // N_CORES  # 2048 rows per core
P = 128
T = RPC // P  # 16 tiles per core
NBUF = 4

_NC = None


def _build():
    nc = bass.Bass()
    f32 = mybir.dt.float32
    x = nc.dram_tensor("x", [RPC, CS], f32, kind="ExternalInput")
    w = nc.dram_tensor("w", [P, T * C], f32, kind="ExternalInput")
    terms = nc.dram_tensor("terms", [P, T], f32, kind="ExternalOutput")

    # Every tile lands in four 1024-column chunks (512 KiB each): the
    # chunked reduce (1.13us) tracks each chunk DMA (1.46us), so DVE never
    # builds a backlog.  The final tile tapers so the very last reduce
    # after the last byte lands is ~340ns.
    def chunks_for(t):
        if t == T - 1:
            widths = [1024, 1024, 1024, 512, 256, 256]
        else:
            widths = [CS // 4] * 4
        cols, c = [], 0
        for wd in widths:
            cols.append((c, wd))
            c += wd
        assert c == CS
        return cols

    max_chunks = max(len(chunks_for(t)) for t in range(T))

    with ExitStack() as ctx:
        xt = ctx.enter_context(nc.sbuf_tensor([P, NBUF * CS], f32))
        w_all = ctx.enter_context(nc.sbuf_tensor([P, T * C], f32))
        cs_all = ctx.enter_context(nc.sbuf_tensor([P, T * C], f32))
        prod_all = ctx.enter_context(nc.sbuf_tensor([P, T * C], f32))
        m_all = ctx.enter_context(nc.sbuf_tensor([P, T], f32))
        margin = ctx.enter_context(nc.sbuf_tensor([P, 1], f32))
        res = ctx.enter_context(nc.sbuf_tensor([P, T], f32))
        # One sem per (buffer, chunk slot): at most ONE outstanding DMA per
        # sem, so a sem value of 16*use_count unambiguously means that use
        # completed (SDMA engines can interleave completions of concurrent
        # DMAs sharing a sem — intermediate counts would be ambiguous).
        x_sems = [
            [
                ctx.enter_context(nc.semaphore(f"dma_x{b}_{k}"))
                for k in range(max_chunks)
            ]
            for b in range(NBUF)
        ]
        dma_o_sem = ctx.enter_context(nc.semaphore("dma_o"))
        dve_sem = ctx.enter_context(nc.semaphore("dve"))
        block = ctx.enter_context(nc.Block())

        @block.sync
        def _(sync):
            # w is only needed by the epilogue mul; it rides sem[0][0]
            # (FIFO on the SP HWDGE ring => it lands before tile 0 chunk 0,
            # and the wait below is for the full issued count on that sem).
            sync.dma_start(w_all[:], w[:]).then_inc(x_sems[0][0], 16)
            for t in range(T):
                if t >= NBUF:
                    # buffer t%NBUF is free once DVE reduced tile t-NBUF
                    sync.wait_ge(dve_sem, t - NBUF + 2)
                buf = t % NBUF
                for i, (col, width) in enumerate(chunks_for(t)):
                    sync.dma_start(
                        xt[:, buf * CS + col : buf * CS + col + width],
                        x[t * P : (t + 1) * P, col : col + width],
                    ).then_inc(x_sems[buf][i], 16)

        @block.vector
        def _(vector):
            vector.memset(margin[:], MARGIN).then_inc(dve_sem, 1)
            counts = [[0] * max_chunks for _ in range(NBUF)]
            counts[0][0] = 1  # the w DMA
            for t in range(T):
                buf = t % NBUF
                chunks = chunks_for(t)
                for i, (col, width) in enumerate(chunks):
                    counts[buf][i] += 1
                    vector.wait_ge(x_sems[buf][i], 16 * counts[buf][i])
                    ins = vector.reduce_sum(
                        cs_all[:, t * C + col // S : t * C + (col + width) // S],
                        xt[
                            :, buf * CS + col : buf * CS + col + width
                        ].rearrange("p (c s) -> p c s", s=S),
                        axis=mybir.AxisListType.X,
                    )
                    if i == len(chunks) - 1:
                        ins.then_inc(dve_sem, 1)  # tile t done -> tick t+2
            vector.drain()  # same-engine RAW: cs_all
            vector.tensor_mul(prod_all[:], cs_all[:], w_all[:])
            vector.drain()  # same-engine RAW: prod_all
            vector.reduce_sum(
                m_all[:],
                prod_all[:].rearrange("p (t c) -> p t c", c=C),
                axis=mybir.AxisListType.X,
            ).then_inc(dve_sem, 1)  # tick T+2

        @block.scalar
        def _(scalar):
            scalar.wait_ge(dve_sem, T + 2)
            # res = relu(-m + MARGIN)
            scalar.activation(
                res[:],
                m_all[:],
                mybir.ActivationFunctionType.Relu,
                bias=margin[:],
                scale=-1.0,
            )
            scalar.drain()  # same-engine RAW: res before HWDGE store
            scalar.dma_start(terms[:], res[:]).then_inc(dma_o_sem, 16)
            scalar.wait_ge(dma_o_sem, 16)

    return nc


def _weights(label, censor):
    """W[b,c] such that pos_mean - neg_mean = sum_c W[b,c]*class_sum[b,c]."""
    lab = np.asarray(label).astype(np.int64)[:, None]  # [B,1]
    cen = np.asarray(censor).astype(np.int64)[:, None]  # [B,1]
    cls = np.arange(C, dtype=np.int64)[None, :]  # [1,C]
    pos = np.where(cen == 0, cls == lab, cls >= lab)  # [B,C] bool
    pos_cnt = pos.sum(1, keepdims=True) * S
    neg_cnt = CS - pos_cnt
    wpos = pos / np.maximum(pos_cnt, 1)
    wneg = (~pos) / np.maximum(neg_cnt, 1)  # rows with neg_cnt==0 have ~pos all False
    return (wpos - wneg).astype(np.float32)


def _in_maps(sim, label, censor):
    W = _weights(label, censor)
    maps = []
    for k in range(N_CORES):
        r0 = k * RPC
        xs = np.ascontiguousarray(sim[r0 : r0 + RPC])
        # w layout on device: w[p, t*C + c] = W[r0 + t*128 + p, c]
        ws = np.ascontiguousarray(
            W[r0 : r0 + RPC].reshape(T, P, C).transpose(1, 0, 2).reshape(P, T * C)
        )
        maps.append({"x": xs, "w": ws})
    return maps


def _get_nc():
    global _NC
    if _NC is None:
        _NC = _build()
    return _NC


def kernel(sim, label, censor, sample_times):
    sim = np.ascontiguousarray(np.asarray(sim, dtype=np.float32))
    assert sim.shape == (B, CS), sim.shape
    assert int(np.asarray(sample_times)) == S
    maps = _in_maps(sim, label, censor)
    res = run_bass_kernel_spmd(_get_nc(), maps, list(range(N_CORES))).results
    terms = np.stack([res[k]["terms"] for k in range(N_CORES)])  # [8,128,16]
    loss = terms.astype(np.float64).mean()
    return np.array(loss, dtype=np.float32)


# revision 13
# speedup vs baseline: 1.0039x; 1.0039x over previous
"""Trainium2 Bass kernel for the BezierSurv censor-margin loss.

Math: for each row b of sim [B, C*S] (C=16 classes, S=256 samples):
  pos/neg masks over the C class segments are fully determined by
  (label[b], censor[b]); both masked means are linear in the per-class
  segment sums.  So
     loss_term[b] = relu(MARGIN - pos_mean + neg_mean)
                  = relu(MARGIN - sum_c W[b,c] * class_sum[b,c])
  with W[b,c] = pos_mask/pos_cnt - neg_mask/neg_cnt (host-precomputed
  [B,16] f32 — tiny), and class_sum the [B,16] segment-reduce of sim —
  the only memory-bound work (256 MiB of HBM reads).

Distribution: pure data parallel over 8 NeuronCores, 2048 rows each.
Per core: 16 row-tiles of [128, 4096]; per tile one DVE 3D-AP segment
reduce into a persistent [128, T*C] cs_all.  The margin dot product
(cs_all * W, 3D reduce) and the ScalarE relu run once at the end, so the
steady-state DVE work per tile (wait + one reduce, ~4.6us) stays under
the 5.8us DMA time per 2 MiB tile.  The last two tiles stream in four
1024-column chunks so the tail reduce overlaps the final DMAs.
Output: per-row relu terms [128,16]; final mean on host.

Raw Bass (no TileContext): explicit 4-buffer DMA pipeline with one
semaphore per buffer slot (unambiguous completion ordering).  SP issues
HWDGE DMAs; DVE reduces; ACT does the relu.
"""

import sys

import numpy as np

for _p in ("/opt/trn_rl_repo",):
    if _p not in sys.path:
        sys.path.insert(0, _p)

from contextlib import ExitStack

import concourse.bass as bass
import concourse.mybir as mybir
from concourse.bass_utils import run_bass_kernel_spmd

MARGIN = 0.1
B = 16384
C = 16
S = 256
CS = C * S
N_CORES = 8
RPC = B // N_CORES  # 2048 rows per core
P = 128
T = RPC // P  # 16 tiles per core
NBUF = 4

_NC = None


def _build():
    nc = bass.Bass()
    f32 = mybir.dt.float32
    x = nc.dram_tensor("x", [RPC, CS], f32, kind="ExternalInput")
    w = nc.dram_tensor("w", [P, T * C], f32, kind="ExternalInput")
    terms = nc.dram_tensor("terms", [P, T], f32, kind="ExternalOutput")

    # Every tile lands in four 1024-column chunks (512 KiB each): the
    # chunked reduce (1.13us) tracks each chunk DMA (1.46us), so DVE never
    # builds a backlog.  The final tile tapers so the very last reduce
    # after the last byte lands is ~340ns.
    def chunks_for(t):
        if t == T - 1:
            widths = [1024, 768, 768, 512, 512, 256, 256]
        else:
            widths = [CS // 4] * 4
        cols, c = [], 0
        for wd in widths:
            cols.append((c, wd))
            c += wd
        assert c == CS
        return cols

    max_chunks = max(len(chunks_for(t)) for t in range(T))

    with ExitStack() as ctx:
        xt = ctx.enter_context(nc.sbuf_tensor([P, NBUF * CS], f32))
        w_all = ctx.enter_context(nc.sbuf_tensor([P, T * C], f32))
        cs_all = ctx.enter_context(nc.sbuf_tensor([P, T * C], f32))
        prod_all = ctx.enter_context(nc.sbuf_tensor([P, T * C], f32))
        m_all = ctx.enter_context(nc.sbuf_tensor([P, T], f32))
        margin = ctx.enter_context(nc.sbuf_tensor([P, 1], f32))
        res = ctx.enter_context(nc.sbuf_tensor([P, T], f32))
        # One sem per (buffer, chunk slot): at most ONE outstanding DMA per
        # sem, so a sem value of 16*use_count unambiguously means that use
        # completed (SDMA engines can interleave completions of concurrent
        # DMAs sharing a sem — intermediate counts would be ambiguous).
        x_sems = [
            [
                ctx.enter_context(nc.semaphore(f"dma_x{b}_{k}"))
                for k in range(max_chunks)
            ]
            for b in range(NBUF)
        ]
        dma_o_sem = ctx.enter_context(nc.semaphore("dma_o"))
        dve_sem = ctx.enter_context(nc.semaphore("dve"))
        block = ctx.enter_context(nc.Block())

        @block.sync
        def _(sync):
            # w is only needed by the epilogue mul; it rides sem[0][0]
            # (FIFO on the SP HWDGE ring => it lands before tile 0 chunk 0,
            # and the wait below is for the full issued count on that sem).
            sync.dma_start(w_all[:], w[:]).then_inc(x_sems[0][0], 16)
            for t in range(T):
                if t >= NBUF:
                    # buffer t%NBUF is free once DVE reduced tile t-NBUF
                    sync.wait_ge(dve_sem, t - NBUF + 2)
                buf = t % NBUF
                for i, (col, width) in enumerate(chunks_for(t)):
                    sync.dma_start(
                        xt[:, buf * CS + col : buf * CS + col + width],
                        x[t * P : (t + 1) * P, col : col + width],
                    ).then_inc(x_sems[buf][i], 16)

        @block.vector
        def _(vector):
            vector.memset(margin[:], MARGIN).then_inc(dve_sem, 1)
            counts = [[0] * max_chunks for _ in range(NBUF)]
            counts[0][0] = 1  # the w DMA
            for t in range(T):
                buf = t % NBUF
                chunks = chunks_for(t)
                for i, (col, width) in enumerate(chunks):
                    counts[buf][i] += 1
                    vector.wait_ge(x_sems[buf][i], 16 * counts[buf][i])
                    ins = vector.reduce_sum(
                        cs_all[:, t * C + col // S : t * C + (col + width) // S],
                        xt[
                            :, buf * CS + col : buf * CS + col + width
                        ].rearrange("p (c s) -> p c s", s=S),
                        axis=mybir.AxisListType.X,
                    )
                    if i == len(chunks) - 1:
                        ins.then_inc(dve_sem, 1)  # tile t done -> tick t+2
            vector.drain()  # same-engine RAW: cs_all
            vector.tensor_mul(prod_all[:], cs_all[:], w_all[:])
            vector.drain()  # same-engine RAW: prod_all
            vector.reduce_sum(
                m_all[:],
                prod_all[:].rearrange("p (t c) -> p t c", c=C),
                axis=mybir.AxisListType.X,
            ).then_inc(dve_sem, 1)  # tick T+2

        @block.scalar
        def _(scalar):
            scalar.wait_ge(dve_sem, T + 2)
            # res = relu(-m + MARGIN)
            scalar.activation(
                res[:],
                m_all[:],
                mybir.ActivationFunctionType.Relu,
                bias=margin[:],
                scale=-1.0,
            )
            scalar.drain()  # same-engine RAW: res before HWDGE store
            scalar.dma_start(terms[:], res[:]).then_inc(dma_o_sem, 16)
            scalar.wait_ge(dma_o_sem, 16)

    return nc


def _weights(label, censor):
    """W[b,c] such that pos_mean - neg_mean = sum_c W[b,c]*class_sum[b,c]."""
    lab = np.asarray(label).astype(np.int64)[:, None]  # [B,1]
    cen = np.asarray(censor).astype(np.int64)[:, None]  # [B,1]
    cls = np.arange(C, dtype=np.int64)[None, :]  # [1,C]
    pos = np.where(cen == 0, cls == lab, cls >= lab)  # [B,C] bool
    pos_cnt = pos.sum(1, keepdims=True) * S
    neg_cnt = CS - pos_cnt
    wpos = pos / np.maximum(pos_cnt, 1)
    wneg = (~pos) / np.maximum(neg_cnt, 1)  # rows with neg_cnt==0 have ~pos all False
    return (wpos - wneg).astype(np.float32)


def _in_maps(sim, label, censor):
    W = _weights(label, censor)
    maps = []
    for k in range(N_CORES):
        r0 = k * RPC
        xs = np.ascontiguousarray(sim[r0 : r0 + RPC])
        # w layout on device: w[p, t*C + c] = W[r0 + t*128 + p, c]
        ws = np.ascontiguousarray(
            W[r0 : r0 + RPC].reshape(T, P, C).transpose(1, 0, 2).reshape(P, T * C)
        )
        maps.append({"x": xs, "w": ws})
    return maps


def _get_nc():
    global _NC
    if _NC is None:
        _NC = _build()
    return _NC


def kernel(sim, label, censor, sample_times):
    sim = np.ascontiguousarray(np.asarray(sim, dtype=np.float32))
    assert sim.shape == (B, CS), sim.shape
    assert int(np.asarray(sample_times)) == S
    maps = _in_maps(sim, label, censor)
    res = run_bass_kernel_spmd(_get_nc(), maps, list(range(N_CORES))).results
    terms = np.stack([res[k]["terms"] for k in range(N_CORES)])  # [8,128,16]
    loss = terms.astype(np.float64).mean()
    return np.array(loss, dtype=np.float32)
